# revision 18
# baseline (speedup 1.0000x reference)
"""Causal self-attention with RoPE for trn2, 8-core SPMD.

Problem (hardcoded): B=2, T=2048, C=1024, 16 heads, head_dim=64, fp32 io.
  qkv = x @ w_attn.T; q,k roped; causal softmax(q k^T/8) v; y @ w_proj.T

Sharding: core c -> (batch b = c//4, head-group g = c%4) — 4 heads per core.
Each core computes its group's partial output projection; host sums the 4
group partials per batch.

Device layout (per core):
  xblk [4*C, 512] bf16 — x[b] transposed, t-block-major (one DMA per block)
  wqkT [C, 512] bf16 — [Wq_g | Wk_g] transposed (cols: 4 heads x 64 q, then k)
  wvT  [C, 260] bf16 — Wv_g transposed, padded: per head 64 cols + 1 zero col
                       (the zero col becomes the "ones" column for sum-exp)
  wpT  [256, C] bf16 — w_proj[:, group cols] transposed
  cosT/sinT [128, T] bf16 — RoPE tables transposed, 2-head stacked; sinT rows
                       0:32/64:96 pre-negated so rope = q*cos + swap32(q)*sin
  mask [128, 128] bf16 — single causal triangle (keep iff f >= p)
  out  [T, C]  f32   — partial output (host sums the 4 group partials)

v2 notes vs baseline:
  - x arrives bf16 in 4 per-block DMAs; per-block SBUF tiles so the first
    projection only waits on the first DMA (~3us, was ~21us).
  - 8 dummy warmup matmuls at t=0 keep the PE HAM busy-window warm so real
    matmuls start at 2.4 GHz instead of 1.2.
  - softmax 1/sum-exp via DVE reciprocal_approx_fast + gpsimd broadcast —
    no more ACT Ln<->Exp table thrash (was 17 x 1.3us reloads).
  - causal masking: memset the fully-masked prefix + one [128,128] triangle
    multiply per head (was [128,1024] mask muls); exp skips the dead prefix.
  - rope rotate-half strip copies moved to gpsimd (DVE was near-saturated).
"""

from contextlib import ExitStack

import numpy as np
import ml_dtypes

import concourse.bass as bass
import concourse.tile as tile
from concourse import bacc, mybir
from concourse.bass_utils import run_bass_kernel_spmd

B, T, C = 2, 2048, 1024
NH, HD = 16, 64
HG = 4              # heads per group (per core)
GD = HG * HD        # 256
NCC = C // 128      # 8 contraction chunks
F32 = mybir.dt.float32
BF16 = mybir.dt.bfloat16
BF = ml_dtypes.bfloat16

QB = 512            # query block size
KT = 128            # key tile size
NTB = T // QB       # 4 t-blocks


def build_kernel(t=T):
    nc = bacc.Bacc("TRN2", target_bir_lowering=False, debug=False)
    xblk = nc.dram_tensor("xblk", [NTB * C, QB], BF16,
                          kind="ExternalInput").ap()
    wqkT = nc.dram_tensor("wqkT", [C, 2 * GD], BF16, kind="ExternalInput").ap()
    wvT = nc.dram_tensor("wvT", [C, HG * (HD + 1)], BF16,
                         kind="ExternalInput").ap()
    wpT = nc.dram_tensor("wpT", [GD, C], BF16, kind="ExternalInput").ap()
    cosT = nc.dram_tensor("cosT", [128, t], BF16, kind="ExternalInput").ap()
    sinT = nc.dram_tensor("sinT", [128, t], BF16, kind="ExternalInput").ap()
    mask = nc.dram_tensor("mask", [128, KT], BF16, kind="ExternalInput").ap()
    out = nc.dram_tensor("out", [t, C], BF16, kind="ExternalOutput").ap()

    with tile.TileContext(nc) as tc:
        _attn_body(tc, out, xblk, wqkT, wvT, wpT, cosT, sinT, mask, t)
    nc.compile()
    return nc


def _attn_body(tc, out, xblk, wqkT, wvT, wpT, cosT, sinT, mask, t):
    ctx = ExitStack()
    nc = tc.nc
    ntt = t // 128          # t tiles (and k tiles)
    nqb = t // QB           # query blocks (== t blocks)
    Exp = mybir.ActivationFunctionType.Exp
    Log = mybir.ActivationFunctionType.Ln

    # Preload the ONE activation table set that contains BOTH exp and ln
    # (natural_log_exp_and_others) so the compiler's table-load inserter
    # never needs to thrash between per-function sets (was 17 x 1.3us).
    from concourse.hw_specs import get_activation_tables
    atl_sets = list(get_activation_tables(nc.m.arch).keys())
    atl_id = atl_sets.index("natural_log_exp_and_others")
    nc.scalar.add_instruction(
        mybir.InstLoadActFuncSet(
            name="manual_atl", ins=[], outs=[], act_func_set_id=atl_id))

    consts = ctx.enter_context(tc.tile_pool(name="consts", bufs=1))
    resident = ctx.enter_context(tc.tile_pool(name="resident", bufs=1))
    ropet = ctx.enter_context(tc.tile_pool(name="ropet", bufs=3))
    exps = ctx.enter_context(tc.tile_pool(name="exps", bufs=10))
    small = ctx.enter_context(tc.tile_pool(name="small", bufs=4))
    outsb = ctx.enter_context(tc.tile_pool(name="outsb", bufs=4))
    psA = ctx.enter_context(tc.tile_pool(name="psA", bufs=2, space="PSUM"))
    psS = ctx.enter_context(tc.tile_pool(name="psS", bufs=2, space="PSUM"))
    psY = ctx.enter_context(tc.tile_pool(name="psY", bufs=2, space="PSUM"))

    # ---- PE warmup: ~3.4us of dummy matmuls releases the HAM clock gate
    # so the first real matmuls run at 2.4 GHz (cold is 1.2). Runs while
    # the first DMAs stream in; the dummy PSUM is never read.
    wz = consts.tile([128, QB], BF16)
    nc.vector.memset(wz[:], 0.0)
    psdum = psA.tile([128, QB], F32, tag="psA")
    for _ in range(8):
        nc.tensor.matmul(psdum[:], wz[:, 0:128], wz[:],
                         start=True, stop=True)

    # ---- constants + x in (one sync queue; order = need order) ----
    wqk_sb = consts.tile([128, NCC, 2 * GD], BF16)
    nc.sync.dma_start(wqk_sb[:], wqkT.rearrange("(cc p) j -> p cc j", p=128))
    xb = []
    for tb in range(nqb):
        xt = resident.tile([128, NCC, QB], BF16, tag=f"x{tb}")
        xb.append(xt)
    nc.sync.dma_start(
        xb[0][:], xblk[0:C, :].rearrange("(cc p) q -> p cc q", p=128))
    cos_sb = consts.tile([128, t], BF16)
    nc.sync.dma_start(cos_sb[:], cosT[:])
    sin_sb = consts.tile([128, t], BF16)
    nc.sync.dma_start(sin_sb[:], sinT[:])
    wv_sb = consts.tile([128, NCC, HG * (HD + 1)], BF16)
    nc.sync.dma_start(wv_sb[:], wvT.rearrange("(cc p) j -> p cc j", p=128))
    nc.sync.dma_start(
        xb[1][:], xblk[C:2 * C, :].rearrange("(cc p) q -> p cc q", p=128))
    mask_sb = consts.tile([128, KT], BF16)
    nc.sync.dma_start(mask_sb[:], mask[:])
    wp_sb = consts.tile([128, 2, C], BF16)
    nc.sync.dma_start(wp_sb[:], wpT.rearrange("(jc p) c -> p jc c", p=128))
    nc.sync.dma_start(
        xb[2][:], xblk[2 * C:3 * C, :].rearrange("(cc p) q -> p cc q", p=128))
    nc.sync.dma_start(
        xb[3][:], xblk[3 * C:4 * C, :].rearrange("(cc p) q -> p cc q", p=128))

    qk = resident.tile([128, 4, t], BF16, tag="qk")
    v_sb = resident.tile([128, ntt * HG, HD + 1], BF16, tag="v")
    ynorm = resident.tile([128, 2, t], BF16, tag="ynorm")

    def qk_proj(jt, tb):
        # qk chunks: 0 = q heads(0,1), 1 = q heads(2,3), 2 = k(0,1), 3 = k(2,3)
        tsl = bass.ts(tb, QB)
        ps = psA.tile([128, QB], F32, tag="psA")
        for cc in range(NCC):
            nc.tensor.matmul(
                ps[:], wqk_sb[:, cc, bass.ts(jt, 128)], xb[tb][:, cc, :],
                start=(cc == 0), stop=(cc == NCC - 1))
            if cc % 2 == 1:
                yield
        raw = ropet.tile([128, QB], BF16, tag="raw")
        nc.vector.tensor_copy(raw[:], ps[:])
        # head dims are interleaved (2j <- old j, 2j+1 <- old j+32) so
        # rotate-half is an adjacent-lane swap: one DVE stream_shuffle
        # (same 32-lane mask in every quadrant) instead of 4 strip copies
        rot = ropet.tile([128, QB], BF16, tag="rot")
        nc.vector.stream_shuffle(rot[:], raw[:], mask=[i ^ 1 for i in range(32)])
        cosp = ropet.tile([128, QB], BF16, tag="cosp")
        nc.vector.tensor_mul(cosp[:], raw[:], cos_sb[:, tsl])
        sinp = ropet.tile([128, QB], BF16, tag="sinp")
        nc.vector.tensor_mul(sinp[:], rot[:], sin_sb[:, tsl])
        # the final add runs on the (otherwise idle) gpsimd engine
        nc.gpsimd.tensor_add(qk[:, jt, tsl], cosp[:], sinp[:])
        yield

    def v_proj(tt):
        # v layout [128, ntt*HG, 65]: (t-tile, local head): 64 cols + 1 ones
        ps = psA.tile([128, HG * (HD + 1)], F32, tag="psA")
        for cc in range(NCC):
            nc.tensor.matmul(
                ps[:], xb[tt // 4][:, cc, bass.ts(tt % 4, 128)], wv_sb[:, cc, :],
                start=(cc == 0), stop=(cc == NCC - 1))
            if cc % 2 == 1:
                yield
        nc.vector.tensor_copy(
            v_sb[:, tt * HG:(tt + 1) * HG, :],
            ps.rearrange("p (h d) -> p h d", d=HD + 1))
        nc.vector.memset(v_sb[:, tt * HG:(tt + 1) * HG, HD], 1.0)
        yield

    def attention_steps(qb, p):
        """Generator yielding one kt-step at a time (software-pipelined:
        S^T/exp for kt runs one step ahead of the V matmuls)."""
        qsl = bass.ts(qb, QB)
        nkt = (qb + 1) * (QB // KT)
        qc = qk[:, p, :]
        kc = qk[:, 2 + p, :]
        ya = psY.tile([HD + 1, QB], F32, tag="psY")
        yb = psY.tile([HD + 1, QB], F32, tag="psY")
        ets = {}
        for kt in range(nkt + 1):
            # V matmuls for the previous kt go first: they are ready (their
            # exp finished a step ago) while S(kt) may still wait on a PSUM
            # slot — keep the PE FIFO unblocked
            if kt >= 1:
                kv = kt - 1
                et = ets.pop(kv)
                first, last = (kv == 0), (kv == nkt - 1)
                # diagonal steps d>=1: queries f < d*KT see none of this key
                # tile — their et region is never written; shrink the AV
                # matmul to the live columns (their ya region simply gets no
                # contribution from this kv, which is the correct sum)
                dv = kv - qb * (QB // KT)
                off = dv * KT if dv > 0 else 0
                nc.tensor.matmul(ya[:, off:QB], v_sb[:, kv * HG + 2 * p, :],
                                 et[:, off:QB], start=first, stop=last)
                nc.tensor.matmul(yb[:, off:QB],
                                 v_sb[:, kv * HG + 2 * p + 1, :],
                                 et[:, QB + off:2 * QB],
                                 start=first, stop=last)
                yield
            if kt < nkt:
                ksl = bass.ts(kt, KT)
                pss = psS.tile([128, 2 * QB], F32, tag="psS")
                nc.tensor.matmul(pss[:, 0:QB], kc[0:64, ksl], qc[0:64, qsl],
                                 start=True, stop=True, tile_position=(0, 0))
                nc.tensor.matmul(pss[:, QB:2 * QB], kc[64:128, ksl],
                                 qc[64:128, qsl],
                                 start=True, stop=True, tile_position=(64, 0))
                et = exps.tile([128, 2 * QB], BF16, tag="exps")
                d = kt - qb * (QB // KT)
                if d <= 0:
                    nc.scalar.activation(et[:], pss[:], Exp, scale=0.125)
                    if d == 0:  # diagonal head tile: triangle mask per head
                        nc.vector.tensor_mul(et[:, 0:KT], et[:, 0:KT],
                                             mask_sb[:])
                        nc.vector.tensor_mul(et[:, QB:QB + KT],
                                             et[:, QB:QB + KT], mask_sb[:])
                else:
                    # diagonal block d>=1: queries f < d*KT see none of this
                    # key tile — skip their exp entirely (the AV matmul also
                    # skips those columns), triangle-mask the boundary strip
                    off = d * KT
                    nc.scalar.activation(et[:, off:2 * QB], pss[:, off:2 * QB],
                                         Exp, scale=0.125)
                    nc.vector.tensor_mul(et[:, off:off + KT],
                                         et[:, off:off + KT], mask_sb[:])
                    nc.vector.tensor_mul(et[:, QB + off:QB + off + KT],
                                         et[:, QB + off:QB + off + KT],
                                         mask_sb[:])
                ets[kt] = et
            yield
        # 1/sumexp = exp(-ln(s)) on ACT — with the combined exp+ln table
        # preloaded this costs no table switches
        lab = small.tile([1, 2 * QB], F32, tag="lab")
        nc.scalar.activation(lab[:, 0:QB], ya[HD:HD + 1, :], Log)
        nc.scalar.activation(lab[:, QB:2 * QB], yb[HD:HD + 1, :], Log)
        rab = small.tile([1, 2 * QB], F32, tag="rab")
        nc.scalar.activation(rab[:], lab[:], Exp, scale=-1.0)
        for h01, yp in ((0, ya), (1, yb)):
            rb = small.tile([64, QB], F32, tag="rb")
            nc.gpsimd.partition_broadcast(rb[:],
                                          rab[:, h01 * QB:(h01 + 1) * QB])
            nc.vector.tensor_mul(ynorm[h01 * 64:(h01 + 1) * 64, p, qsl],
                                 yp[0:HD, :], rb[:])
        yield

    def out_proj(tt, cb):
        ps = psA.tile([128, QB], F32, tag="psA")
        for jc in range(2):
            nc.tensor.matmul(
                ps[:], ynorm[:, jc, bass.ts(tt, 128)],
                wp_sb[:, jc, bass.ts(cb, QB)],
                start=(jc == 0), stop=(jc == 1))
        ot = outsb.tile([128, QB], BF16, tag="ot")
        nc.vector.tensor_copy(ot[:], ps[:])
        nc.sync.dma_start(
            out[tt * 128:(tt + 1) * 128, bass.ts(cb, QB)], ot[:])
        yield

    # ---- interleaved schedule ----
    # Projection for block tb feeds attention for qb=tb (causal attention
    # needs K/V only up to the diagonal).  Attention's kt-steps for block tb
    # are woven with small granules of the *next* block's projection and the
    # *previous* block's output projection, so the PE always has a little
    # independent matmul work queued while ACT chews through exp, without
    # long FIFO chains delaying the next S^T matmul.
    from collections import deque
    proj_fill = deque()   # next block's qk/v projection granules
    out_fill = deque()    # completed blocks' output-projection granules

    def drain(n, last_block=False):
        for _ in range(n):
            if proj_fill:
                q = proj_fill
            elif out_fill and (last_block or len(out_fill) > 8):
                # hold ~8 output-projection granules in reserve so the final
                # block's attention (which has no next-block projections to
                # weave) still has independent PE work
                q = out_fill
            else:
                return
            g = q.popleft()
            try:
                next(g)
                q.append(g)
            except StopIteration:
                pass

    def force(q):
        while q:
            g = q.popleft()
            for _ in g:
                pass

    # p=0's heads (jt 0 and 2) first so the first S matmuls unblock sooner
    for jt in (0, 2, 1, 3):
        for _ in qk_proj(jt, 0):
            pass
    for tt in range(4):
        for _ in v_proj(tt):
            pass
    for tb in range(nqb):
        force(proj_fill)  # attention(tb) needs block tb's projections traced
        if tb + 1 < nqb:
            for jt in (0, 2, 1, 3):
                proj_fill.append(qk_proj(jt, tb + 1))
            for tt in range((tb + 1) * 4, (tb + 1) * 4 + 4):
                proj_fill.append(v_proj(tt))
        last = (tb == nqb - 1)
        for p in range(2):
            for _ in attention_steps(tb, p):
                drain(2 if tb < 2 else 1, last_block=last)
        for tt in range(tb * 4, tb * 4 + 4):
            for cb in range(2):
                out_fill.append(out_proj(tt, cb))
    force(proj_fill)
    force(out_fill)
    ctx.close()


def host_inputs(x, w_attn, w_proj, t=T):
    """Build the 8 per-core input maps from full inputs."""
    xblks = []
    for b in range(B):
        blk = np.concatenate(
            [np.ascontiguousarray(x[b, tb * QB:(tb + 1) * QB, :].T)
             for tb in range(t // QB)], axis=0)
        xblks.append(blk.astype(BF))
    inv = 1.0 / (10000.0 ** (np.arange(0, HD, 2, dtype=np.float32) / HD))
    fr = np.outer(np.arange(t, dtype=np.float32), inv)     # [t, 32]
    emb = np.concatenate([fr, fr], 1)                      # [t, 64]
    cos = np.cos(emb).T.astype(np.float32)                 # [64, t]
    sin = np.sin(emb).T.astype(np.float32)
    sin_s = sin.copy()
    sin_s[:32] *= -1.0
    # interleaved head-dim order: device row 2j <- rope dim j, 2j+1 <- j+32,
    # so rotate-half on device is an adjacent-lane swap (stream_shuffle)
    perm = np.empty(HD, np.int64)
    perm[0::2] = np.arange(HD // 2)
    perm[1::2] = np.arange(HD // 2, HD)
    cosT2 = np.tile(cos[perm], (2, 1)).astype(BF)
    sinT2 = np.tile(sin_s[perm], (2, 1)).astype(BF)

    # single causal triangle tile: keep iff f >= p
    f = np.arange(KT)[None, :]
    pp = np.arange(KT)[:, None]
    mask = (f >= pp).astype(BF)                            # [128, 128]

    # per-head row permutation applying the interleaved dim order to q/k
    hperm = np.concatenate([h * HD + perm for h in range(HG)])

    in_maps = []
    for c in range(8):
        b, g = c // 4, c % 4
        wq = w_attn[g * GD:(g + 1) * GD][hperm]
        wk = w_attn[C + g * GD:C + (g + 1) * GD][hperm]
        wv = w_attn[2 * C + g * GD:2 * C + (g + 1) * GD]
        wqkT = np.ascontiguousarray(
            np.concatenate([wq, wk], 0).T).astype(BF)
        wvT = np.zeros((C, HG * (HD + 1)), BF)
        for h in range(HG):
            wvT[:, h * (HD + 1):h * (HD + 1) + HD] = \
                wv[h * HD:(h + 1) * HD].T.astype(BF)
        wpT = np.ascontiguousarray(
            w_proj[:, g * GD:(g + 1) * GD].T).astype(BF)
        in_maps.append({"xblk": xblks[b], "wqkT": wqkT, "wvT": wvT,
                        "wpT": wpT, "cosT": cosT2, "sinT": sinT2,
                        "mask": mask})
    return in_maps


_cache = {}


def kernel(x, w_attn, w_proj):
    x = np.asarray(x, dtype=np.float32)
    w_attn = np.asarray(w_attn, dtype=np.float32)
    w_proj = np.asarray(w_proj, dtype=np.float32)
    if "nc" not in _cache:
        _cache["nc"] = build_kernel()
    nc = _cache["nc"]
    in_maps = host_inputs(x, w_attn, w_proj)
    res = run_bass_kernel_spmd(nc, in_maps, list(range(8)))
    out = np.zeros((B, T, C), dtype=np.float32)
    for c in range(8):
        out[c // 4] += res.results[c]["out"].astype(np.float32)
    return out


# revision 19
# speedup vs baseline: 1.2769x; 1.2769x over previous
"""Causal self-attention with RoPE for trn2, 8-core SPMD.

Problem (hardcoded): B=2, T=2048, C=1024, 16 heads, head_dim=64, fp32 io.
  qkv = x @ w_attn.T; q,k roped; causal softmax(q k^T/8) v; y @ w_proj.T

Sharding: core c -> (batch b = c//4, head-group g = c%4) — 4 heads per core.
Each core computes its group's partial output projection; host sums the 4
group partials per batch.

Device layout (per core):
  xblk [4*C, 512] bf16 — x[b] transposed, t-block-major (one DMA per block)
  wqkT [C, 512] bf16 — [Wq_g | Wk_g] transposed (cols: 4 heads x 64 q, then k)
  wvT  [C, 260] bf16 — Wv_g transposed, padded: per head 64 cols + 1 zero col
                       (the zero col becomes the "ones" column for sum-exp)
  wpT  [256, C] bf16 — w_proj[:, group cols] transposed
  cosT/sinT [128, T] bf16 — RoPE tables transposed, 2-head stacked; sinT rows
                       0:32/64:96 pre-negated so rope = q*cos + swap32(q)*sin
  mask [128, 128] bf16 — single causal triangle (keep iff f >= p)
  out  [T, C]  f32   — partial output (host sums the 4 group partials)

v2 notes vs baseline:
  - x arrives bf16 in 4 per-block DMAs; per-block SBUF tiles so the first
    projection only waits on the first DMA (~3us, was ~21us).
  - 8 dummy warmup matmuls at t=0 keep the PE HAM busy-window warm so real
    matmuls start at 2.4 GHz instead of 1.2.
  - softmax 1/sum-exp via DVE reciprocal_approx_fast + gpsimd broadcast —
    no more ACT Ln<->Exp table thrash (was 17 x 1.3us reloads).
  - causal masking: memset the fully-masked prefix + one [128,128] triangle
    multiply per head (was [128,1024] mask muls); exp skips the dead prefix.
  - rope rotate-half strip copies moved to gpsimd (DVE was near-saturated).
"""

from contextlib import ExitStack

import numpy as np
import ml_dtypes

import concourse.bass as bass
import concourse.tile as tile
from concourse import bacc, mybir
from concourse.bass_utils import run_bass_kernel_spmd

B, T, C = 2, 2048, 1024
NH, HD = 16, 64
HG = 4              # heads per group (per core)
GD = HG * HD        # 256
NCC = C // 128      # 8 contraction chunks
F32 = mybir.dt.float32
BF16 = mybir.dt.bfloat16
BF = ml_dtypes.bfloat16

QB = 512            # query block size
KT = 128            # key tile size
NTB = T // QB       # 4 t-blocks


def build_kernel(t=T):
    nc = bacc.Bacc("TRN2", target_bir_lowering=False, debug=False)
    xblk = nc.dram_tensor("xblk", [NTB * C, QB], BF16,
                          kind="ExternalInput").ap()
    wqkT = nc.dram_tensor("wqkT", [C, 2 * GD], BF16, kind="ExternalInput").ap()
    wvT = nc.dram_tensor("wvT", [C, HG * (HD + 1)], BF16,
                         kind="ExternalInput").ap()
    wpT = nc.dram_tensor("wpT", [GD, C], BF16, kind="ExternalInput").ap()
    cosT = nc.dram_tensor("cosT", [128, t], BF16, kind="ExternalInput").ap()
    sinT = nc.dram_tensor("sinT", [128, t], BF16, kind="ExternalInput").ap()
    mask = nc.dram_tensor("mask", [128, KT], BF16, kind="ExternalInput").ap()
    out = nc.dram_tensor("out", [t, C], BF16, kind="ExternalOutput").ap()

    with tile.TileContext(nc) as tc:
        _attn_body(tc, out, xblk, wqkT, wvT, wpT, cosT, sinT, mask, t)
    nc.compile()
    return nc


def _attn_body(tc, out, xblk, wqkT, wvT, wpT, cosT, sinT, mask, t):
    ctx = ExitStack()
    nc = tc.nc
    ntt = t // 128          # t tiles (and k tiles)
    nqb = t // QB           # query blocks (== t blocks)
    Exp = mybir.ActivationFunctionType.Exp
    Log = mybir.ActivationFunctionType.Ln

    # Preload the ONE activation table set that contains BOTH exp and ln
    # (natural_log_exp_and_others) so the compiler's table-load inserter
    # never needs to thrash between per-function sets (was 17 x 1.3us).
    from concourse.hw_specs import get_activation_tables
    atl_sets = list(get_activation_tables(nc.m.arch).keys())
    atl_id = atl_sets.index("natural_log_exp_and_others")
    nc.scalar.add_instruction(
        mybir.InstLoadActFuncSet(
            name="manual_atl", ins=[], outs=[], act_func_set_id=atl_id))

    consts = ctx.enter_context(tc.tile_pool(name="consts", bufs=1))
    resident = ctx.enter_context(tc.tile_pool(name="resident", bufs=1))
    ropet = ctx.enter_context(tc.tile_pool(name="ropet", bufs=3))
    exps = ctx.enter_context(tc.tile_pool(name="exps", bufs=10))
    small = ctx.enter_context(tc.tile_pool(name="small", bufs=4))
    outsb = ctx.enter_context(tc.tile_pool(name="outsb", bufs=4))
    psA = ctx.enter_context(tc.tile_pool(name="psA", bufs=2, space="PSUM"))
    psS = ctx.enter_context(tc.tile_pool(name="psS", bufs=2, space="PSUM"))
    psY = ctx.enter_context(tc.tile_pool(name="psY", bufs=2, space="PSUM"))

    # ---- PE warmup: ~3.4us of dummy matmuls releases the HAM clock gate
    # so the first real matmuls run at 2.4 GHz (cold is 1.2). Runs while
    # the first DMAs stream in; the dummy PSUM is never read.
    wz = consts.tile([128, QB], BF16)
    nc.vector.memset(wz[:], 0.0)
    psdum = psA.tile([128, QB], F32, tag="psA")
    for _ in range(8):
        nc.tensor.matmul(psdum[:], wz[:, 0:128], wz[:],
                         start=True, stop=True)

    # ---- constants + x in (one sync queue; order = need order) ----
    wqk_sb = consts.tile([128, NCC, 2 * GD], BF16)
    nc.sync.dma_start(wqk_sb[:], wqkT.rearrange("(cc p) j -> p cc j", p=128))
    xb = []
    for tb in range(nqb):
        xt = resident.tile([128, NCC, QB], BF16, tag=f"x{tb}")
        xb.append(xt)
    nc.sync.dma_start(
        xb[0][:], xblk[0:C, :].rearrange("(cc p) q -> p cc q", p=128))
    cos_sb = consts.tile([128, t], BF16)
    nc.sync.dma_start(cos_sb[:], cosT[:])
    sin_sb = consts.tile([128, t], BF16)
    nc.sync.dma_start(sin_sb[:], sinT[:])
    wv_sb = consts.tile([128, NCC, HG * (HD + 1)], BF16)
    nc.sync.dma_start(wv_sb[:], wvT.rearrange("(cc p) j -> p cc j", p=128))
    nc.sync.dma_start(
        xb[1][:], xblk[C:2 * C, :].rearrange("(cc p) q -> p cc q", p=128))
    mask_sb = consts.tile([128, KT], BF16)
    nc.sync.dma_start(mask_sb[:], mask[:])
    wp_sb = consts.tile([128, 2, C], BF16)
    nc.sync.dma_start(wp_sb[:], wpT.rearrange("(jc p) c -> p jc c", p=128))
    nc.sync.dma_start(
        xb[2][:], xblk[2 * C:3 * C, :].rearrange("(cc p) q -> p cc q", p=128))
    nc.sync.dma_start(
        xb[3][:], xblk[3 * C:4 * C, :].rearrange("(cc p) q -> p cc q", p=128))

    qk = resident.tile([128, 4, t], BF16, tag="qk")
    v_sb = resident.tile([128, ntt * HG, HD + 1], BF16, tag="v")
    ynorm = resident.tile([128, 2, t], BF16, tag="ynorm")

    def qk_proj(jt, tb):
        # qk chunks: 0 = q heads(0,1), 1 = q heads(2,3), 2 = k(0,1), 3 = k(2,3)
        tsl = bass.ts(tb, QB)
        ps = psA.tile([128, QB], F32, tag="psA")
        for cc in range(NCC):
            nc.tensor.matmul(
                ps[:], wqk_sb[:, cc, bass.ts(jt, 128)], xb[tb][:, cc, :],
                start=(cc == 0), stop=(cc == NCC - 1))
            if cc % 2 == 1:
                yield
        raw = ropet.tile([128, QB], BF16, tag="raw")
        nc.vector.tensor_copy(raw[:], ps[:])
        # head dims are interleaved (2j <- old j, 2j+1 <- old j+32) so
        # rotate-half is an adjacent-lane swap: one DVE stream_shuffle
        # (same 32-lane mask in every quadrant) instead of 4 strip copies
        rot = ropet.tile([128, QB], BF16, tag="rot")
        nc.vector.stream_shuffle(rot[:], raw[:], mask=[i ^ 1 for i in range(32)])
        cosp = ropet.tile([128, QB], BF16, tag="cosp")
        nc.vector.tensor_mul(cosp[:], raw[:], cos_sb[:, tsl])
        sinp = ropet.tile([128, QB], BF16, tag="sinp")
        nc.vector.tensor_mul(sinp[:], rot[:], sin_sb[:, tsl])
        nc.vector.tensor_add(qk[:, jt, tsl], cosp[:], sinp[:])
        yield

    def v_proj(tt):
        # v layout [128, ntt*HG, 65]: (t-tile, local head): 64 cols + 1 ones
        ps = psA.tile([128, HG * (HD + 1)], F32, tag="psA")
        for cc in range(NCC):
            nc.tensor.matmul(
                ps[:], xb[tt // 4][:, cc, bass.ts(tt % 4, 128)], wv_sb[:, cc, :],
                start=(cc == 0), stop=(cc == NCC - 1))
            if cc % 2 == 1:
                yield
        nc.vector.tensor_copy(
            v_sb[:, tt * HG:(tt + 1) * HG, :],
            ps.rearrange("p (h d) -> p h d", d=HD + 1))
        nc.vector.memset(v_sb[:, tt * HG:(tt + 1) * HG, HD], 1.0)
        yield

    def attention_steps(qb, p):
        """Generator yielding one kt-step at a time (software-pipelined:
        S^T/exp for kt runs one step ahead of the V matmuls)."""
        qsl = bass.ts(qb, QB)
        nkt = (qb + 1) * (QB // KT)
        qc = qk[:, p, :]
        kc = qk[:, 2 + p, :]
        ya = psY.tile([HD + 1, QB], F32, tag="psY")
        yb = psY.tile([HD + 1, QB], F32, tag="psY")
        ets = {}
        for kt in range(nkt + 1):
            # V matmuls for the previous kt go first: they are ready (their
            # exp finished a step ago) while S(kt) may still wait on a PSUM
            # slot — keep the PE FIFO unblocked
            if kt >= 1:
                kv = kt - 1
                et = ets.pop(kv)
                first, last = (kv == 0), (kv == nkt - 1)
                # diagonal steps d>=1: queries f < d*KT see none of this key
                # tile — their et region is never written; shrink the AV
                # matmul to the live columns (their ya region simply gets no
                # contribution from this kv, which is the correct sum)
                dv = kv - qb * (QB // KT)
                off = dv * KT if dv > 0 else 0
                nc.tensor.matmul(ya[:, off:QB], v_sb[:, kv * HG + 2 * p, :],
                                 et[:, off:QB], start=first, stop=last)
                nc.tensor.matmul(yb[:, off:QB],
                                 v_sb[:, kv * HG + 2 * p + 1, :],
                                 et[:, QB + off:2 * QB],
                                 start=first, stop=last)
                yield
            if kt < nkt:
                ksl = bass.ts(kt, KT)
                pss = psS.tile([128, 2 * QB], F32, tag="psS")
                nc.tensor.matmul(pss[:, 0:QB], kc[0:64, ksl], qc[0:64, qsl],
                                 start=True, stop=True, tile_position=(0, 0))
                nc.tensor.matmul(pss[:, QB:2 * QB], kc[64:128, ksl],
                                 qc[64:128, qsl],
                                 start=True, stop=True, tile_position=(64, 0))
                et = exps.tile([128, 2 * QB], BF16, tag="exps")
                d = kt - qb * (QB // KT)
                if d <= 0:
                    nc.scalar.activation(et[:], pss[:], Exp, scale=0.125)
                    if d == 0:  # diagonal head tile: triangle mask per head
                        nc.vector.tensor_mul(et[:, 0:KT], et[:, 0:KT],
                                             mask_sb[:])
                        nc.vector.tensor_mul(et[:, QB:QB + KT],
                                             et[:, QB:QB + KT], mask_sb[:])
                else:
                    # diagonal block d>=1: queries f < d*KT see none of this
                    # key tile — skip their exp entirely (the AV matmul also
                    # skips those columns), triangle-mask the boundary strip
                    off = d * KT
                    nc.scalar.activation(et[:, off:2 * QB], pss[:, off:2 * QB],
                                         Exp, scale=0.125)
                    nc.vector.tensor_mul(et[:, off:off + KT],
                                         et[:, off:off + KT], mask_sb[:])
                    nc.vector.tensor_mul(et[:, QB + off:QB + off + KT],
                                         et[:, QB + off:QB + off + KT],
                                         mask_sb[:])
                ets[kt] = et
            yield
        # 1/sumexp = exp(-ln(s)) on ACT — with the combined exp+ln table
        # preloaded this costs no table switches
        lab = small.tile([1, 2 * QB], F32, tag="lab")
        nc.scalar.activation(lab[:, 0:QB], ya[HD:HD + 1, :], Log)
        nc.scalar.activation(lab[:, QB:2 * QB], yb[HD:HD + 1, :], Log)
        rab = small.tile([1, 2 * QB], F32, tag="rab")
        nc.scalar.activation(rab[:], lab[:], Exp, scale=-1.0)
        for h01, yp in ((0, ya), (1, yb)):
            rb = small.tile([64, QB], F32, tag="rb")
            nc.gpsimd.partition_broadcast(rb[:],
                                          rab[:, h01 * QB:(h01 + 1) * QB])
            nc.vector.tensor_mul(ynorm[h01 * 64:(h01 + 1) * 64, p, qsl],
                                 yp[0:HD, :], rb[:])
        yield

    def out_proj(tt, cb):
        ps = psA.tile([128, QB], F32, tag="psA")
        for jc in range(2):
            nc.tensor.matmul(
                ps[:], ynorm[:, jc, bass.ts(tt, 128)],
                wp_sb[:, jc, bass.ts(cb, QB)],
                start=(jc == 0), stop=(jc == 1))
        ot = outsb.tile([128, QB], BF16, tag="ot")
        nc.vector.tensor_copy(ot[:], ps[:])
        nc.sync.dma_start(
            out[tt * 128:(tt + 1) * 128, bass.ts(cb, QB)], ot[:])
        yield

    # ---- interleaved schedule ----
    # Projection for block tb feeds attention for qb=tb (causal attention
    # needs K/V only up to the diagonal).  Attention's kt-steps for block tb
    # are woven with small granules of the *next* block's projection and the
    # *previous* block's output projection, so the PE always has a little
    # independent matmul work queued while ACT chews through exp, without
    # long FIFO chains delaying the next S^T matmul.
    from collections import deque
    proj_fill = deque()   # next block's qk/v projection granules
    out_fill = deque()    # completed blocks' output-projection granules

    def drain(n, last_block=False):
        for _ in range(n):
            if proj_fill:
                q = proj_fill
            elif out_fill and (last_block or len(out_fill) > 8):
                # hold ~8 output-projection granules in reserve so the final
                # block's attention (which has no next-block projections to
                # weave) still has independent PE work
                q = out_fill
            else:
                return
            g = q.popleft()
            try:
                next(g)
                q.append(g)
            except StopIteration:
                pass

    def force(q):
        while q:
            g = q.popleft()
            for _ in g:
                pass

    # p=0's heads (jt 0 and 2) first so the first S matmuls unblock sooner
    for jt in (0, 2, 1, 3):
        for _ in qk_proj(jt, 0):
            pass
    for tt in range(4):
        for _ in v_proj(tt):
            pass
    for tb in range(nqb):
        force(proj_fill)  # attention(tb) needs block tb's projections traced
        if tb + 1 < nqb:
            for jt in (0, 2, 1, 3):
                proj_fill.append(qk_proj(jt, tb + 1))
            for tt in range((tb + 1) * 4, (tb + 1) * 4 + 4):
                proj_fill.append(v_proj(tt))
        last = (tb == nqb - 1)
        for p in range(2):
            for _ in attention_steps(tb, p):
                drain(2 if tb < 2 else 1, last_block=last)
        for tt in range(tb * 4, tb * 4 + 4):
            for cb in range(2):
                out_fill.append(out_proj(tt, cb))
    force(proj_fill)
    force(out_fill)
    ctx.close()


def host_inputs(x, w_attn, w_proj, t=T):
    """Build the 8 per-core input maps from full inputs."""
    xblks = []
    for b in range(B):
        blk = np.concatenate(
            [np.ascontiguousarray(x[b, tb * QB:(tb + 1) * QB, :].T)
             for tb in range(t // QB)], axis=0)
        xblks.append(blk.astype(BF))
    inv = 1.0 / (10000.0 ** (np.arange(0, HD, 2, dtype=np.float32) / HD))
    fr = np.outer(np.arange(t, dtype=np.float32), inv)     # [t, 32]
    emb = np.concatenate([fr, fr], 1)                      # [t, 64]
    cos = np.cos(emb).T.astype(np.float32)                 # [64, t]
    sin = np.sin(emb).T.astype(np.float32)
    sin_s = sin.copy()
    sin_s[:32] *= -1.0
    # interleaved head-dim order: device row 2j <- rope dim j, 2j+1 <- j+32,
    # so rotate-half on device is an adjacent-lane swap (stream_shuffle)
    perm = np.empty(HD, np.int64)
    perm[0::2] = np.arange(HD // 2)
    perm[1::2] = np.arange(HD // 2, HD)
    cosT2 = np.tile(cos[perm], (2, 1)).astype(BF)
    sinT2 = np.tile(sin_s[perm], (2, 1)).astype(BF)

    # single causal triangle tile: keep iff f >= p
    f = np.arange(KT)[None, :]
    pp = np.arange(KT)[:, None]
    mask = (f >= pp).astype(BF)                            # [128, 128]

    # per-head row permutation applying the interleaved dim order to q/k
    hperm = np.concatenate([h * HD + perm for h in range(HG)])

    in_maps = []
    for c in range(8):
        b, g = c // 4, c % 4
        wq = w_attn[g * GD:(g + 1) * GD][hperm]
        wk = w_attn[C + g * GD:C + (g + 1) * GD][hperm]
        wv = w_attn[2 * C + g * GD:2 * C + (g + 1) * GD]
        wqkT = np.ascontiguousarray(
            np.concatenate([wq, wk], 0).T).astype(BF)
        wvT = np.zeros((C, HG * (HD + 1)), BF)
        for h in range(HG):
            wvT[:, h * (HD + 1):h * (HD + 1) + HD] = \
                wv[h * HD:(h + 1) * HD].T.astype(BF)
        wpT = np.ascontiguousarray(
            w_proj[:, g * GD:(g + 1) * GD].T).astype(BF)
        in_maps.append({"xblk": xblks[b], "wqkT": wqkT, "wvT": wvT,
                        "wpT": wpT, "cosT": cosT2, "sinT": sinT2,
                        "mask": mask})
    return in_maps


_cache = {}


def kernel(x, w_attn, w_proj):
    x = np.asarray(x, dtype=np.float32)
    w_attn = np.asarray(w_attn, dtype=np.float32)
    w_proj = np.asarray(w_proj, dtype=np.float32)
    if "nc" not in _cache:
        _cache["nc"] = build_kernel()
    nc = _cache["nc"]
    in_maps = host_inputs(x, w_attn, w_proj)
    res = run_bass_kernel_spmd(nc, in_maps, list(range(8)))
    out = np.zeros((B, T, C), dtype=np.float32)
    for c in range(8):
        out[c // 4] += res.results[c]["out"].astype(np.float32)
    return out
